# revision 12
# baseline (speedup 1.0000x reference)
"""Trainium2 Bass kernel for nn_CLF_block (channel-attention block), v2.

Reference computation (per batch item, x = concat([a,b], ch) in [256, N],
N = H*W = 16384):
    z  = w1 x + b1 1^T
    q  = w2 z + b2 1^T ;  k = w3 z + b3 1^T ;  v = w4 z + b4 1^T
    qk = q k^T ; attn = softmax(qk, -1) ; out = attn v

Algebraic restructuring (as v1): only two O(C^2 N) passes over x, the rest
is 256x256 algebra:
    Gx = x x^T ; sx = x 1          (pass 1, one streaming pass)
    ... tiny algebra -> attn -> W = attn w4 w1, c0 ...
    out = W x + c0 1^T             (pass 2, one streaming pass)

v2 changes vs v1:
  * pass 1 is a SINGLE f32r Gram (f32r matmul with moving dim >= 256 runs
    at 1 cyc/col on the PE), replacing the fp16 hi/lo split double-Gram.
    Per-product precision ~2^-17 -- better than the split scheme needs.
  * fully streamed: x is never resident; transposed f32r x (16.8 MiB)
    streams through pass 1, a separate fp16 copy (8.4 MiB) streams
    through pass 2, output leaves as fp16 (8 MiB, host upcasts).
    Total HBM traffic 34.5 MiB/core vs 49.5 in v1.
  * DMA: input pieces alternate across the sync/scalar queues, consts ride
    the gpsimd queue, outputs the gpsimd queue; pass-2 psum drains
    alternate between scalar and vector so neither engine paces the PE.

Sharding: data-parallel over batch, one batch item per NeuronCore (B=8).
"""

import sys

if "/opt/trn_rl_repo" not in sys.path:
    sys.path.insert(0, "/opt/trn_rl_repo")

from contextlib import ExitStack

import numpy as np

import concourse.bass as bass
import concourse.mybir as mybir
import concourse.tile as tile
from concourse import bacc
from concourse.bass_utils import run_bass_kernel_spmd

F32 = mybir.dt.float32
F32R = mybir.dt.float32r
F16 = mybir.dt.float16
P = 128           # partitions / channel block
C = 256           # channels
NPIX = 128 * 128  # spatial positions per batch item
NPIECE = 16       # xrt stream pieces
PIECE = NPIX // NPIECE   # 1024 n per piece
CH_PP = PIECE // P       # 8 gram chunks per piece
NCHUNK = NPIX // P       # 128 gram chunks
NJ = 4            # pass-2 column superpieces
JW = NPIX // NJ   # 4096 cols per superpiece
NT = 512          # pass-2 psum tile width
TPJ = JW // NT    # 8 psum tiles per (j, b)
CW = C + 2        # gram stream width: 256 ch + ones col + even-pad col


def _emit(nc, tc, ctx, d_in, d_out):
    """Emit the Tile program for one core (one batch item)."""
    acat, ident = d_in["acat"], d_in["ident"]
    xrt_d, xh_d = d_in["xrt"], d_in["xh"]
    brows, bcols = d_in["brows"], d_in["bcols"]
    out_d = d_out["out"]

    const = ctx.enter_context(tc.tile_pool(name="const", bufs=1))

    # --- constants (gpsimd queue; small, streams alongside pass-1 input) --
    # acat = [A2^T | A3^T | A4] with A_i = w_i w1 folded on the host.
    a_sb = []
    for k in range(2):
        at = const.tile([P, 3 * C], F32R, name=f"a_sb{k}", tag=f"a_sb{k}")
        nc.gpsimd.dma_start(out=at, in_=acat[k * P:(k + 1) * P, :])
        a_sb.append(at)
    a2t = [a_sb[k][:, 0 * C:1 * C] for k in range(2)]   # A2^T [cin, o2]
    a3t = [a_sb[k][:, 1 * C:2 * C] for k in range(2)]   # A3^T [cin, o3]
    a4r = [a_sb[k][:, 2 * C:3 * C] for k in range(2)]   # A4   [d,   c]

    rows = []
    for r in range(3):
        rt = const.tile([1, C], F32, name=f"brow{r}", tag=f"brow{r}")
        nc.gpsimd.dma_start(out=rt, in_=brows[r:r + 1, :])
        rows.append(rt)
    c3_row, nc3_row, c2_row = rows

    bc_sb = []
    for k in range(2):
        bt = const.tile([P, 1], F32, name=f"bcol{k}", tag=f"bcol{k}")
        nc.gpsimd.dma_start(out=bt, in_=bcols[k * P:(k + 1) * P, :])
        bc_sb.append(bt)
    c4_col = [bc_sb[k][:, 0:1] for k in range(2)]

    ident_sb = const.tile([P, P], F32R, name="ident_sb", tag="ident_sb")
    nc.gpsimd.dma_start(out=ident_sb, in_=ident[:, :])

    # --- pass 1: Gx = x x^T as a single streamed f32r Gram ----------------
    # xrt piece layout [P(n), CH_PP, CW]; column 256 is ones, so psum
    # column 256 accumulates sx = x 1 (exact row sums of f32r-rounded x);
    # column 257 is zero padding (fp32r matmul needs an even free size).
    gx_f = [
        const.tile([P, C + 1], F32, name=f"gx_f{b}", tag=f"gx_f{b}")
        for b in range(2)
    ]
    with tc.tile_pool(name="gx_ps", bufs=1, space="PSUM") as gxp, \
         tc.tile_pool(name="xrt_sb", bufs=3) as xtp:
        shh_ps = [
            gxp.tile([P, CW], F32, name=f"shh_ps{b}", tag=f"shh{b}")
            for b in range(2)
        ]
        for i in range(NPIECE):
            xrt_p = xtp.tile([P, CH_PP, CW], F32R, name="xrt_p",
                             tag="xrt_p")
            (nc.sync, nc.scalar, nc.gpsimd)[i % 3].dma_start(
                out=xrt_p, in_=xrt_d[i])
            for g in range(CH_PP):
                ch = i * CH_PP + g
                for b in range(2):
                    nc.tensor.matmul(shh_ps[b],
                                     xrt_p[:, g, b * P:(b + 1) * P],
                                     xrt_p[:, g, :],
                                     start=(ch == 0),
                                     stop=(ch == NCHUNK - 1))
        for b in range(2):
            nc.vector.tensor_copy(gx_f[b], shh_ps[b][:, 0:C + 1])

    # --- pass-2 input stream (issued now so the queues drain it right
    # after the xrt pieces; all NJ tiles live at once -> no buffer gating).
    xh_pool = ctx.enter_context(tc.tile_pool(name="xh_sb", bufs=1))
    xh_sb = [[None] * NJ for _ in range(2)]
    for k in range(2):
        for j in range(NJ):
            xt = xh_pool.tile([P, JW], F16, name=f"xh{k}_{j}",
                              tag=f"xh{k}_{j}")
            (nc.sync if k == 0 else nc.scalar).dma_start(
                out=xt, in_=xh_d[k, :, j * JW:(j + 1) * JW])
            xh_sb[k][j] = xt

    # Split the (large) diagonal out of Gx: products (Gx-D) A2^T are much
    # smaller, so the f32r tile rounding no longer pollutes qk; the exact
    # diagonal contribution re-enters via the E = A2^T * d term below.
    gxd = []
    for b in range(2):
        bs = slice(b * P, (b + 1) * P)
        dm = const.tile([P, P], F32, name=f"gxdm{b}", tag=f"gxdm{b}")
        nc.vector.tensor_mul(dm, gx_f[b][:, bs], ident_sb.bitcast(F32))
        dcol = const.tile([P, 1], F32, name=f"gxd{b}", tag=f"gxd{b}")
        nc.vector.reduce_sum(out=dcol, in_=dm, axis=mybir.AxisListType.X)
        nc.vector.tensor_sub(gx_f[b][:, bs], gx_f[b][:, bs], dm)
        gxd.append(dcol)

    # --- tiny 256x256 algebra ------------------------------------------
    #   qk = A2 Gx A3^T + p2 c3^T + c2 p3^T + N c2 c3^T   (p_i = A_i sx)
    #   attn = softmax(qk) ; W = attn A4 ; c0 = attn c4
    # Wide products run as f32r (explicitly cast operands); rank-1 and
    # [*,1]-shaped products stay fp32 in separate psum tiles.
    alg_sb = const

    with tc.tile_pool(name="alg_ps", bufs=4, space="PSUM") as ap, \
         tc.tile_pool(name="tp_ps", bufs=2, space="PSUM") as tpp:
        # E = A2^T scaled per-partition by the Gx diagonal
        e_sb = []
        for k in range(2):
            ef = alg_sb.tile([P, C], F32, name=f"e_f{k}", tag=f"e_f{k}")
            nc.vector.tensor_scalar_mul(ef, a2t[k].bitcast(F32), gxd[k])
            e_sb.append(ef)

        # p2 = (A2 sx)^T, p3 = (A3 sx)^T  (fp32 rank-ish, off critical path)
        p2_row = alg_sb.tile([1, C], F32, name="p2_row", tag="p2_row")
        p3_row = alg_sb.tile([1, C], F32, name="p3_row", tag="p3_row")
        for dst, at_ in ((p2_row, a2t), (p3_row, a3t)):
            pps = ap.tile([1, C], F32, name="pps", tag="alg")
            for k in range(2):
                nc.tensor.matmul(pps, gx_f[k][:, C:C + 1],
                                 at_[k].bitcast(F32),
                                 start=(k == 0), stop=(k == 1))
            nc.vector.tensor_copy(dst, pps)

        # rank-1 qk terms (fp32, own psum per block)
        rk_ps = []
        for b in range(2):
            rps = ap.tile([P, C], F32, name=f"rk{b}", tag=f"rk{b}",
                          bufs=1)
            nc.tensor.matmul(rps, p2_row[:, b * P:(b + 1) * P], c3_row,
                             start=True, stop=False)
            nc.tensor.matmul(rps, c2_row[:, b * P:(b + 1) * P], p3_row,
                             start=False, stop=False)
            nc.tensor.matmul(rps, c2_row[:, b * P:(b + 1) * P], nc3_row,
                             start=False, stop=True)
            rkf = alg_sb.tile([P, C], F32, name=f"rk_f{b}", tag=f"rk_f{b}")
            nc.scalar.activation(out=rkf, in_=rps,
                                 func=mybir.ActivationFunctionType.Copy)
            rk_ps.append(rkf)

        # Y = ((Gx - D) A2^T)^T : lhsT = Gx' blocks, rhs = A2^T  (f32r)
        y_sb = []
        for b in range(2):
            yps = ap.tile([P, C], F32, name=f"y{b}", tag="alg")
            for k in range(2):
                nc.tensor.matmul(yps, gx_f[k][:, b * P:(b + 1) * P],
                                 a2t[k].bitcast(F32),
                                 start=(k == 0), stop=(k == 1))
            yt = alg_sb.tile([P, C], F32, name=f"y_sb{b}", tag=f"y_sb{b}")
            nc.vector.tensor_copy(yt, yps)
            y_sb.append(yt)

        # qk = Y^T A3^T + E^T A3^T (diag term) ; then + rank-1s ; softmax
        attn_sb = []
        for b in range(2):
            qkps = ap.tile([P, C], F32, name=f"qk{b}", tag="alg")
            for k in range(2):
                nc.tensor.matmul(qkps, y_sb[k][:, b * P:(b + 1) * P],
                                 a3t[k].bitcast(F32),
                                 start=(k == 0), stop=False)
            for k in range(2):
                nc.tensor.matmul(qkps, e_sb[k][:, b * P:(b + 1) * P],
                                 a3t[k].bitcast(F32),
                                 start=False, stop=(k == 1))
            qk_f = alg_sb.tile([P, C], F32, name=f"qk_f{b}", tag=f"qk_f{b}")
            nc.vector.tensor_add(qk_f, qkps, rk_ps[b])

            negmax = alg_sb.tile([P, 1], F32, name=f"negmax{b}", tag=f"nm{b}")
            nc.vector.tensor_reduce(
                out=negmax, in_=qk_f, op=mybir.AluOpType.max,
                axis=mybir.AxisListType.X, negate=True,
            )
            expq = alg_sb.tile([P, C], F32, name=f"expq{b}", tag=f"expq{b}")
            nc.scalar.activation(
                out=expq, in_=qk_f, func=mybir.ActivationFunctionType.Exp,
                bias=negmax, scale=1.0,
            )
            denom = alg_sb.tile([P, 1], F32, name=f"denom{b}", tag=f"dn{b}")
            nc.vector.reduce_sum(out=denom, in_=expq,
                                 axis=mybir.AxisListType.X)
            rden = alg_sb.tile([P, 1], F32, name=f"rden{b}", tag=f"rd{b}")
            nc.vector.reciprocal(rden, denom)
            at_ = alg_sb.tile([P, C], F32, name=f"attn{b}", tag=f"attn{b}")
            nc.vector.tensor_scalar_mul(at_, expq, rden)
            attn_sb.append(at_)

        # attn^T (4 PE transposes, cast to f32r on drain)
        attnT_sb = [
            alg_sb.tile([P, C], F32R, name=f"attnT{j}", tag=f"attnT{j}")
            for j in range(2)
        ]
        for b in range(2):
            for j in range(2):
                tps = tpp.tile([P, P], F32, name="tps", tag="algtp")
                nc.tensor.transpose(tps,
                                    attn_sb[b][:, j * P:(j + 1) * P],
                                    ident_sb.bitcast(F32))
                nc.vector.tensor_copy(attnT_sb[j][:, b * P:(b + 1) * P], tps)

        # W^T = A4-as-lhsT @ attn^T (f32r) ; cast fp16 for pass 2
        wt_sb = []
        for b in range(2):
            wps = ap.tile([P, C], F32, name=f"w{b}", tag="alg")
            for k in range(2):
                nc.tensor.matmul(wps, a4r[k][:, b * P:(b + 1) * P],
                                 attnT_sb[k], start=(k == 0), stop=(k == 1))
            wt_ = alg_sb.tile([P, C], F16, name=f"wt_sb{b}", tag=f"wt_sb{b}")
            nc.vector.tensor_copy(wt_, wps)
            wt_sb.append(wt_)

        # c0_col = attn c4 (per block, fp32)
        c0_col = []
        for b in range(2):
            cps = ap.tile([P, 1], F32, name=f"c0{b}", tag="alg")
            for k in range(2):
                nc.tensor.matmul(cps,
                                 attnT_sb[k][:, b * P:(b + 1) * P].bitcast(F32),
                                 c4_col[k], start=(k == 0),
                                 stop=(k == 1))
            ct = alg_sb.tile([P, 1], F32, name=f"c0_col{b}", tag=f"c0_col{b}")
            nc.vector.tensor_copy(ct, cps)
            c0_col.append(ct)

    # --- pass 2: out = W xh + c0 1^T, fp16 in/out ------------------------
    # psum drains alternate scalar/vector so neither engine paces the PE;
    # finished [128, JW] staging tiles leave on the gpsimd queue.
    with tc.tile_pool(name="o_ps", bufs=4, space="PSUM") as ops, \
         tc.tile_pool(name="o_sb", bufs=4) as osb:
        for j in range(NJ):
            for b in range(2):
                ot = osb.tile([P, JW], F16, name="ot", tag="ot")
                for t in range(TPJ):
                    pst = ops.tile([P, NT], F32, name="pst", tag="pst")
                    for k in range(2):
                        nc.tensor.matmul(
                            pst,
                            wt_sb[k][:, b * P:(b + 1) * P],
                            xh_sb[k][j][:, t * NT:(t + 1) * NT],
                            start=(k == 0),
                            stop=(k == 1),
                        )
                    osl = ot[:, t * NT:(t + 1) * NT]
                    if t % 2 == 0:
                        nc.scalar.activation(
                            out=osl, in_=pst,
                            func=mybir.ActivationFunctionType.Identity,
                            bias=c0_col[b], scale=1.0,
                        )
                    else:
                        nc.vector.tensor_scalar_add(osl, pst, c0_col[b])
                (nc.gpsimd if b == 0 else nc.sync).dma_start(
                    out=out_d[b * P:(b + 1) * P, j * JW:(j + 1) * JW],
                    in_=ot,
                )


def build_program(enable_asserts=False):
    nc = bacc.Bacc(
        "TRN2",
        target_bir_lowering=False,
        debug=False,
        enable_asserts=enable_asserts,
        num_devices=8,
    )
    d_in = {
        "xrt": nc.dram_tensor("xrt", [NPIECE, P, CH_PP, CW], F32R,
                              kind="ExternalInput").ap(),
        "xh": nc.dram_tensor("xh", [2, P, NPIX], F16,
                             kind="ExternalInput").ap(),
        "acat": nc.dram_tensor("acat", [C, 3 * C], F32R,
                               kind="ExternalInput").ap(),
        "brows": nc.dram_tensor("brows", [3, C], F32,
                                kind="ExternalInput").ap(),
        "bcols": nc.dram_tensor("bcols", [C, 1], F32,
                                kind="ExternalInput").ap(),
        "ident": nc.dram_tensor("ident", [P, P], F32R,
                                kind="ExternalInput").ap(),
    }
    d_out = {
        "out": nc.dram_tensor("out", [C, NPIX], F16,
                              kind="ExternalOutput").ap(),
    }
    with tile.TileContext(nc) as tc, ExitStack() as ctx:
        _emit(nc, tc, ctx, d_in, d_out)
    nc.compile()
    return nc


def _round_f32r(x):
    """Round fp32 to the FP32R-representable set (hi-bf16 + lo-bf16)."""
    import ml_dtypes

    x = np.asarray(x, np.float32)
    hi = x.astype(ml_dtypes.bfloat16).astype(np.float32)
    lo = (x - hi).astype(ml_dtypes.bfloat16).astype(np.float32)
    return hi + lo


def make_in_maps(a, b, w1, b1, w2, b2, w3, b3, w4, b4):
    N = NPIX
    f = np.float32
    w1d, w2d, w3d, w4d = (np.asarray(w, np.float64)
                          for w in (w1, w2, w3, w4))
    b1d = np.asarray(b1, np.float64)
    a2 = (w2d @ w1d).astype(f)
    a3 = (w3d @ w1d).astype(f)
    a4 = (w4d @ w1d).astype(f)
    c2 = (w2d @ b1d + b2).astype(f)
    c3 = (w3d @ b1d + b3).astype(f)
    c4 = (w4d @ b1d + b4).astype(f)
    acat = _round_f32r(np.concatenate([a2.T, a3.T, a4], axis=1))
    brows = np.stack([c3, N * c3, c2]).astype(f)
    bcols = c4[:, None].astype(f)
    ident = np.eye(P, dtype=f)
    B = a.shape[0]
    in_maps = []
    for i in range(B):
        x = np.concatenate([a[i].reshape(P, N), b[i].reshape(P, N)], axis=0)
        xt = _round_f32r(x.T)                       # [N, C]
        xt = np.concatenate([xt, np.ones((N, 1), f),
                             np.zeros((N, 1), f)], axis=1)  # [N, CW]
        xrt = np.ascontiguousarray(
            xt.reshape(NPIECE, CH_PP, P, CW).transpose(0, 2, 1, 3))
        xh = np.ascontiguousarray(
            x.astype(np.float16).reshape(2, P, N))
        in_maps.append({
            "xrt": xrt,
            "xh": xh,
            "acat": acat,
            "brows": brows,
            "bcols": bcols,
            "ident": ident,
        })
    return in_maps


_CACHE = {}


def kernel(a, b, w1, b1, w2, b2, w3, b3, w4, b4, _trace=False):
    a = np.asarray(a, dtype=np.float32)
    b = np.asarray(b, dtype=np.float32)
    args = [np.asarray(t, dtype=np.float32)
            for t in (w1, b1, w2, b2, w3, b3, w4, b4)]
    if "nc" not in _CACHE:
        _CACHE["nc"] = build_program()
    nc = _CACHE["nc"]
    in_maps = make_in_maps(a, b, *args)
    res = run_bass_kernel_spmd(nc, in_maps, core_ids=list(range(8)),
                               trace=_trace)
    B, Ch, H, W = a.shape
    out = np.stack([r["out"].astype(np.float32).reshape(C, H, W)
                    for r in res.results])
    if _trace:
        _CACHE["last_results"] = res
    return out


# revision 13
# speedup vs baseline: 1.0596x; 1.0596x over previous
"""Trainium2 Bass kernel for nn_CLF_block (channel-attention block), v2.

Reference computation (per batch item, x = concat([a,b], ch) in [256, N],
N = H*W = 16384):
    z  = w1 x + b1 1^T
    q  = w2 z + b2 1^T ;  k = w3 z + b3 1^T ;  v = w4 z + b4 1^T
    qk = q k^T ; attn = softmax(qk, -1) ; out = attn v

Algebraic restructuring (as v1): only two O(C^2 N) passes over x, the rest
is 256x256 algebra:
    Gx = x x^T ; sx = x 1          (pass 1, one streaming pass)
    ... tiny algebra -> attn -> W = attn w4 w1, c0 ...
    out = W x + c0 1^T             (pass 2, one streaming pass)

v2 changes vs v1:
  * pass 1 is a SINGLE f32r Gram (f32r matmul with moving dim >= 256 runs
    at 1 cyc/col on the PE), replacing the fp16 hi/lo split double-Gram.
    Per-product precision ~2^-17 -- better than the split scheme needs.
  * fully streamed: x is never resident; transposed f32r x (16.8 MiB)
    streams through pass 1, a separate fp16 copy (8.4 MiB) streams
    through pass 2, output leaves as fp16 (8 MiB, host upcasts).
    Total HBM traffic 34.5 MiB/core vs 49.5 in v1.
  * DMA: input pieces alternate across the sync/scalar queues, consts ride
    the gpsimd queue, outputs the gpsimd queue; pass-2 psum drains
    alternate between scalar and vector so neither engine paces the PE.

Sharding: data-parallel over batch, one batch item per NeuronCore (B=8).
"""

import sys

if "/opt/trn_rl_repo" not in sys.path:
    sys.path.insert(0, "/opt/trn_rl_repo")

from contextlib import ExitStack

import numpy as np

import concourse.bass as bass
import concourse.mybir as mybir
import concourse.tile as tile
from concourse import bacc
from concourse.bass_utils import run_bass_kernel_spmd

F32 = mybir.dt.float32
F32R = mybir.dt.float32r
F16 = mybir.dt.float16
P = 128           # partitions / channel block
C = 256           # channels
NPIX = 128 * 128  # spatial positions per batch item
NPIECE = 16       # xrt stream pieces
PIECE = NPIX // NPIECE   # 1024 n per piece
CH_PP = PIECE // P       # 8 gram chunks per piece
NCHUNK = NPIX // P       # 128 gram chunks
NJ = 4            # pass-2 column superpieces
JW = NPIX // NJ   # 4096 cols per superpiece
NT = 512          # pass-2 psum tile width
TPJ = JW // NT    # 8 psum tiles per (j, b)
CW = C + 2        # gram stream width: 256 ch + ones col + even-pad col


def _emit(nc, tc, ctx, d_in, d_out):
    """Emit the Tile program for one core (one batch item)."""
    acat, ident = d_in["acat"], d_in["ident"]
    xrt_d, xh_d = d_in["xrt"], d_in["xh"]
    brows, bcols = d_in["brows"], d_in["bcols"]
    out_d = d_out["out"]

    const = ctx.enter_context(tc.tile_pool(name="const", bufs=1))

    # --- constants (gpsimd queue; small, streams alongside pass-1 input) --
    # acat = [A2^T | A3^T | A4] with A_i = w_i w1 folded on the host.
    a_sb = []
    for k in range(2):
        at = const.tile([P, 3 * C], F32R, name=f"a_sb{k}", tag=f"a_sb{k}")
        nc.gpsimd.dma_start(out=at, in_=acat[k * P:(k + 1) * P, :])
        a_sb.append(at)
    a2t = [a_sb[k][:, 0 * C:1 * C] for k in range(2)]   # A2^T [cin, o2]
    a3t = [a_sb[k][:, 1 * C:2 * C] for k in range(2)]   # A3^T [cin, o3]
    a4r = [a_sb[k][:, 2 * C:3 * C] for k in range(2)]   # A4   [d,   c]

    rows = []
    for r in range(3):
        rt = const.tile([1, C], F32, name=f"brow{r}", tag=f"brow{r}")
        nc.gpsimd.dma_start(out=rt, in_=brows[r:r + 1, :])
        rows.append(rt)
    c3_row, nc3_row, c2_row = rows

    bc_sb = []
    for k in range(2):
        bt = const.tile([P, 1], F32, name=f"bcol{k}", tag=f"bcol{k}")
        nc.gpsimd.dma_start(out=bt, in_=bcols[k * P:(k + 1) * P, :])
        bc_sb.append(bt)
    c4_col = [bc_sb[k][:, 0:1] for k in range(2)]

    ident_sb = const.tile([P, P], F32R, name="ident_sb", tag="ident_sb")
    nc.gpsimd.dma_start(out=ident_sb, in_=ident[:, :])

    # --- pass 1: Gx = x x^T as a single streamed f32r Gram ----------------
    # xrt piece layout [P(n), CH_PP, CW]; column 256 is ones, so psum
    # column 256 accumulates sx = x 1 (exact row sums of f32r-rounded x);
    # column 257 is zero padding (fp32r matmul needs an even free size).
    gx_f = [
        const.tile([P, C + 1], F32, name=f"gx_f{b}", tag=f"gx_f{b}")
        for b in range(2)
    ]
    with tc.tile_pool(name="gx_ps", bufs=1, space="PSUM") as gxp, \
         tc.tile_pool(name="xrt_sb", bufs=3) as xtp:
        shh_ps = [
            gxp.tile([P, CW], F32, name=f"shh_ps{b}", tag=f"shh{b}")
            for b in range(2)
        ]
        for i in range(NPIECE):
            xrt_p = xtp.tile([P, CH_PP, CW], F32R, name="xrt_p",
                             tag="xrt_p")
            (nc.sync, nc.scalar, nc.gpsimd)[i % 3].dma_start(
                out=xrt_p, in_=xrt_d[i])
            for g in range(CH_PP):
                ch = i * CH_PP + g
                for b in range(2):
                    nc.tensor.matmul(shh_ps[b],
                                     xrt_p[:, g, b * P:(b + 1) * P],
                                     xrt_p[:, g, :],
                                     start=(ch == 0),
                                     stop=(ch == NCHUNK - 1))
        for b in range(2):
            nc.vector.tensor_copy(gx_f[b], shh_ps[b][:, 0:C + 1])

    # --- pass-2 input stream (issued now so the queues drain it right
    # after the xrt pieces; all NJ tiles live at once -> no buffer gating).
    xh_pool = ctx.enter_context(tc.tile_pool(name="xh_sb", bufs=1))
    xh_sb = [[None] * NJ for _ in range(2)]
    for k in range(2):
        for j in range(NJ):
            xt = xh_pool.tile([P, JW], F16, name=f"xh{k}_{j}",
                              tag=f"xh{k}_{j}")
            (nc.sync if k == 0 else nc.scalar).dma_start(
                out=xt, in_=xh_d[k, :, j * JW:(j + 1) * JW])
            xh_sb[k][j] = xt

    # Split the (large) diagonal out of Gx: products (Gx-D) A2^T are much
    # smaller, so the f32r tile rounding no longer pollutes qk; the exact
    # diagonal contribution re-enters via the E = A2^T * d term below.
    gxd = []
    for b in range(2):
        bs = slice(b * P, (b + 1) * P)
        dm = const.tile([P, P], F32, name=f"gxdm{b}", tag=f"gxdm{b}")
        nc.vector.tensor_mul(dm, gx_f[b][:, bs], ident_sb.bitcast(F32))
        dcol = const.tile([P, 1], F32, name=f"gxd{b}", tag=f"gxd{b}")
        nc.vector.reduce_sum(out=dcol, in_=dm, axis=mybir.AxisListType.X)
        nc.vector.tensor_sub(gx_f[b][:, bs], gx_f[b][:, bs], dm)
        gxd.append(dcol)

    # --- tiny 256x256 algebra ------------------------------------------
    #   qk = A2 Gx A3^T + p2 c3^T + c2 p3^T + N c2 c3^T   (p_i = A_i sx)
    #   attn = softmax(qk) ; W = attn A4 ; c0 = attn c4
    # Wide products run as f32r (explicitly cast operands); rank-1 and
    # [*,1]-shaped products stay fp32 in separate psum tiles.
    alg_sb = const

    with tc.tile_pool(name="alg_ps", bufs=4, space="PSUM") as ap, \
         tc.tile_pool(name="tp_ps", bufs=2, space="PSUM") as tpp:
        # E = A2^T scaled per-partition by the Gx diagonal
        e_sb = []
        for k in range(2):
            ef = alg_sb.tile([P, C], F32, name=f"e_f{k}", tag=f"e_f{k}")
            nc.vector.tensor_scalar_mul(ef, a2t[k].bitcast(F32), gxd[k])
            e_sb.append(ef)

        # p2 = (A2 sx)^T, p3 = (A3 sx)^T  (fp32 rank-ish, off critical path)
        p2_row = alg_sb.tile([1, C], F32, name="p2_row", tag="p2_row")
        p3_row = alg_sb.tile([1, C], F32, name="p3_row", tag="p3_row")
        for dst, at_ in ((p2_row, a2t), (p3_row, a3t)):
            pps = ap.tile([1, C], F32, name="pps", tag="alg")
            for k in range(2):
                nc.tensor.matmul(pps, gx_f[k][:, C:C + 1],
                                 at_[k].bitcast(F32),
                                 start=(k == 0), stop=(k == 1))
            nc.vector.tensor_copy(dst, pps)

        # rank-1 qk terms (fp32, own psum per block)
        rk_ps = []
        for b in range(2):
            rps = ap.tile([P, C], F32, name=f"rk{b}", tag=f"rk{b}",
                          bufs=1)
            nc.tensor.matmul(rps, p2_row[:, b * P:(b + 1) * P], c3_row,
                             start=True, stop=False)
            nc.tensor.matmul(rps, c2_row[:, b * P:(b + 1) * P], p3_row,
                             start=False, stop=False)
            nc.tensor.matmul(rps, c2_row[:, b * P:(b + 1) * P], nc3_row,
                             start=False, stop=True)
            rkf = alg_sb.tile([P, C], F32, name=f"rk_f{b}", tag=f"rk_f{b}")
            nc.scalar.activation(out=rkf, in_=rps,
                                 func=mybir.ActivationFunctionType.Copy)
            rk_ps.append(rkf)

        # Y = ((Gx - D) A2^T)^T : lhsT = Gx' blocks, rhs = A2^T  (f32r)
        y_sb = []
        for b in range(2):
            yps = ap.tile([P, C], F32, name=f"y{b}", tag="alg")
            for k in range(2):
                nc.tensor.matmul(yps, gx_f[k][:, b * P:(b + 1) * P],
                                 a2t[k].bitcast(F32),
                                 start=(k == 0), stop=(k == 1))
            yt = alg_sb.tile([P, C], F32, name=f"y_sb{b}", tag=f"y_sb{b}")
            nc.vector.tensor_copy(yt, yps)
            y_sb.append(yt)

        # qk = Y^T A3^T + E^T A3^T (diag term) ; then + rank-1s ; softmax
        attn_sb = []
        for b in range(2):
            qkps = ap.tile([P, C], F32, name=f"qk{b}", tag="alg")
            for k in range(2):
                nc.tensor.matmul(qkps, y_sb[k][:, b * P:(b + 1) * P],
                                 a3t[k].bitcast(F32),
                                 start=(k == 0), stop=False)
            for k in range(2):
                nc.tensor.matmul(qkps, e_sb[k][:, b * P:(b + 1) * P],
                                 a3t[k].bitcast(F32),
                                 start=False, stop=(k == 1))
            qk_f = alg_sb.tile([P, C], F32, name=f"qk_f{b}", tag=f"qk_f{b}")
            nc.vector.tensor_add(qk_f, qkps, rk_ps[b])

            negmax = alg_sb.tile([P, 1], F32, name=f"negmax{b}", tag=f"nm{b}")
            nc.vector.tensor_reduce(
                out=negmax, in_=qk_f, op=mybir.AluOpType.max,
                axis=mybir.AxisListType.X, negate=True,
            )
            expq = alg_sb.tile([P, C], F32, name=f"expq{b}", tag=f"expq{b}")
            nc.scalar.activation(
                out=expq, in_=qk_f, func=mybir.ActivationFunctionType.Exp,
                bias=negmax, scale=1.0,
            )
            denom = alg_sb.tile([P, 1], F32, name=f"denom{b}", tag=f"dn{b}")
            nc.vector.reduce_sum(out=denom, in_=expq,
                                 axis=mybir.AxisListType.X)
            rden = alg_sb.tile([P, 1], F32, name=f"rden{b}", tag=f"rd{b}")
            nc.vector.reciprocal(rden, denom)
            at_ = alg_sb.tile([P, C], F32, name=f"attn{b}", tag=f"attn{b}")
            nc.vector.tensor_scalar_mul(at_, expq, rden)
            attn_sb.append(at_)

        # attn^T (4 PE transposes, cast to f32r on drain)
        attnT_sb = [
            alg_sb.tile([P, C], F32R, name=f"attnT{j}", tag=f"attnT{j}")
            for j in range(2)
        ]
        for b in range(2):
            for j in range(2):
                tps = tpp.tile([P, P], F32, name="tps", tag="algtp")
                nc.tensor.transpose(tps,
                                    attn_sb[b][:, j * P:(j + 1) * P],
                                    ident_sb.bitcast(F32))
                nc.vector.tensor_copy(attnT_sb[j][:, b * P:(b + 1) * P], tps)

        # W^T = A4-as-lhsT @ attn^T (f32r) ; cast fp16 for pass 2
        wt_sb = []
        for b in range(2):
            wps = ap.tile([P, C], F32, name=f"w{b}", tag="alg")
            for k in range(2):
                nc.tensor.matmul(wps, a4r[k][:, b * P:(b + 1) * P].bitcast(F32),
                                 attnT_sb[k].bitcast(F32),
                                 start=(k == 0), stop=(k == 1))
            wt_ = alg_sb.tile([P, C], F16, name=f"wt_sb{b}", tag=f"wt_sb{b}")
            nc.vector.tensor_copy(wt_, wps)
            wt_sb.append(wt_)

        # c0_col = attn c4 (per block, fp32)
        c0_col = []
        for b in range(2):
            cps = ap.tile([P, 1], F32, name=f"c0{b}", tag="alg")
            for k in range(2):
                nc.tensor.matmul(cps,
                                 attnT_sb[k][:, b * P:(b + 1) * P].bitcast(F32),
                                 c4_col[k], start=(k == 0),
                                 stop=(k == 1))
            ct = alg_sb.tile([P, 1], F32, name=f"c0_col{b}", tag=f"c0_col{b}")
            nc.vector.tensor_copy(ct, cps)
            c0_col.append(ct)

    # --- pass 2: out = W xh + c0 1^T, fp16 in/out ------------------------
    # psum drains alternate scalar/vector so neither engine paces the PE;
    # finished [128, JW] staging tiles leave on the gpsimd queue.
    with tc.tile_pool(name="o_ps", bufs=4, space="PSUM") as ops, \
         tc.tile_pool(name="o_sb", bufs=4) as osb:
        for j in range(NJ):
            for b in range(2):
                ot = osb.tile([P, JW], F16, name="ot", tag="ot")
                for t in range(TPJ):
                    pst = ops.tile([P, NT], F32, name="pst", tag="pst")
                    for k in range(2):
                        nc.tensor.matmul(
                            pst,
                            wt_sb[k][:, b * P:(b + 1) * P],
                            xh_sb[k][j][:, t * NT:(t + 1) * NT],
                            start=(k == 0),
                            stop=(k == 1),
                        )
                    osl = ot[:, t * NT:(t + 1) * NT]
                    if t % 2 == 0:
                        nc.scalar.activation(
                            out=osl, in_=pst,
                            func=mybir.ActivationFunctionType.Identity,
                            bias=c0_col[b], scale=1.0,
                        )
                    else:
                        nc.vector.tensor_scalar_add(osl, pst, c0_col[b])
                (nc.gpsimd if b == 0 else nc.sync).dma_start(
                    out=out_d[b * P:(b + 1) * P, j * JW:(j + 1) * JW],
                    in_=ot,
                )


def build_program(enable_asserts=False):
    nc = bacc.Bacc(
        "TRN2",
        target_bir_lowering=False,
        debug=False,
        enable_asserts=enable_asserts,
        num_devices=8,
    )
    d_in = {
        "xrt": nc.dram_tensor("xrt", [NPIECE, P, CH_PP, CW], F32R,
                              kind="ExternalInput").ap(),
        "xh": nc.dram_tensor("xh", [2, P, NPIX], F16,
                             kind="ExternalInput").ap(),
        "acat": nc.dram_tensor("acat", [C, 3 * C], F32R,
                               kind="ExternalInput").ap(),
        "brows": nc.dram_tensor("brows", [3, C], F32,
                                kind="ExternalInput").ap(),
        "bcols": nc.dram_tensor("bcols", [C, 1], F32,
                                kind="ExternalInput").ap(),
        "ident": nc.dram_tensor("ident", [P, P], F32R,
                                kind="ExternalInput").ap(),
    }
    d_out = {
        "out": nc.dram_tensor("out", [C, NPIX], F16,
                              kind="ExternalOutput").ap(),
    }
    with tile.TileContext(nc) as tc, ExitStack() as ctx:
        _emit(nc, tc, ctx, d_in, d_out)
    nc.compile()
    return nc


def _round_f32r(x):
    """Round fp32 to the FP32R-representable set (hi-bf16 + lo-bf16)."""
    import ml_dtypes

    x = np.asarray(x, np.float32)
    hi = x.astype(ml_dtypes.bfloat16).astype(np.float32)
    lo = (x - hi).astype(ml_dtypes.bfloat16).astype(np.float32)
    return hi + lo


def make_in_maps(a, b, w1, b1, w2, b2, w3, b3, w4, b4):
    N = NPIX
    f = np.float32
    w1d, w2d, w3d, w4d = (np.asarray(w, np.float64)
                          for w in (w1, w2, w3, w4))
    b1d = np.asarray(b1, np.float64)
    a2 = (w2d @ w1d).astype(f)
    a3 = (w3d @ w1d).astype(f)
    a4 = (w4d @ w1d).astype(f)
    c2 = (w2d @ b1d + b2).astype(f)
    c3 = (w3d @ b1d + b3).astype(f)
    c4 = (w4d @ b1d + b4).astype(f)
    acat = _round_f32r(np.concatenate([a2.T, a3.T, a4], axis=1))
    brows = np.stack([c3, N * c3, c2]).astype(f)
    bcols = c4[:, None].astype(f)
    ident = np.eye(P, dtype=f)
    B = a.shape[0]
    in_maps = []
    for i in range(B):
        x = np.concatenate([a[i].reshape(P, N), b[i].reshape(P, N)], axis=0)
        xt = _round_f32r(x.T)                       # [N, C]
        xt = np.concatenate([xt, np.ones((N, 1), f),
                             np.zeros((N, 1), f)], axis=1)  # [N, CW]
        xrt = np.ascontiguousarray(
            xt.reshape(NPIECE, CH_PP, P, CW).transpose(0, 2, 1, 3))
        xh = np.ascontiguousarray(
            x.astype(np.float16).reshape(2, P, N))
        in_maps.append({
            "xrt": xrt,
            "xh": xh,
            "acat": acat,
            "brows": brows,
            "bcols": bcols,
            "ident": ident,
        })
    return in_maps


_CACHE = {}


def kernel(a, b, w1, b1, w2, b2, w3, b3, w4, b4, _trace=False):
    a = np.asarray(a, dtype=np.float32)
    b = np.asarray(b, dtype=np.float32)
    args = [np.asarray(t, dtype=np.float32)
            for t in (w1, b1, w2, b2, w3, b3, w4, b4)]
    if "nc" not in _CACHE:
        _CACHE["nc"] = build_program()
    nc = _CACHE["nc"]
    in_maps = make_in_maps(a, b, *args)
    res = run_bass_kernel_spmd(nc, in_maps, core_ids=list(range(8)),
                               trace=_trace)
    B, Ch, H, W = a.shape
    out = np.stack([r["out"].astype(np.float32).reshape(C, H, W)
                    for r in res.results])
    if _trace:
        _CACHE["last_results"] = res
    return out
